# revision 16
# baseline (speedup 1.0000x reference)
"""Trainium2 Bass kernel for the per-task embedding MLP (embedding_lookup).

Computation (per sample j with task t = task_ids[j]):
    h      = x[j] @ l1_emb[t].reshape(256, 128) + l1_bias[t]
    g      = gelu_exact(h)
    out[j] = sum(g * l2_emb[t]) + l2_bias[t, 0]

Strategy: shard the *task* axis across the 8 cores (125 contiguous tasks per
core), so each core streams a contiguous slab of l1_emb exactly once (the
memory roofline), instead of gathering a 128 KiB row per sample (4x more
traffic).  Samples are routed (host-side index math only) to the core owning
their task and packed into a fixed slot grid of W=8 columns per group (tasks
with more than W samples get extra groups with duplicated weight rows), so
all 8 cores run one identical SPMD program.  W=8 rather than 16: with ~4.1
samples/task the group count is nearly unchanged (so PE cost is the same)
but the x slot grid and the whole vector/scalar epilogue halve; only ~3
tasks/core overflow 8 samples and duplicate their 64 KiB w1 row.

Dataflow (v10):
  * The whole per-core w1 slab (NG tasks x 512 B/partition fp16, ~8 MiB)
    lives in ONE persistent SBUF tile.  All chunk DMAs are issued
    back-to-back on the sync HWDGE ring at kernel start (ramp [1, 8] so
    the first matmul starts ~1.5 us in, 32-task chunks in the middle —
    a finer ramp-in costs ~2 us of issue-rate idle — and a [16, 4, 2, 1]
    taper so the final matmuls are not gated on a megabyte receipt).  With no pool recycling the 16 DMA
    engines stream the slab at the ~370 GB/s per-core HBM ceiling and
    matmuls chase per-chunk completion semaphores (subtile deps).
  * x (fp16, [2,128,NSLOT]) and b1/w2 ride the gpsimd SWDGE queue so the
    sync ring carries nothing but the w1 stream.
  * Per PSUM block (<=16 groups): two K=128 fp16 matmuls per group
    accumulate hT[128, cols]; Vector does hs = ps + b1 (column-broadcast
    STT), Scalar does gelu (ACT table), Vector multiplies by w2 into a
    per-SEGMENT fp16 prodt tile.  The hidden-dim reduction (ones-vector
    matmul) runs once per <=512-col SEGMENT, not per block — a per-block
    reduce would make the in-order PE stream wait on the full
    STT->gelu->TT chain every block (~2.6 us/block convoy, the dominant
    cost of earlier revisions).  Segment results are copied PSUM->SBUF on
    the scalar engine and DMA'd out on the (by then drained) sync ring;
    the copy+DMA of segment s are emitted inside segment s+1 so no
    engine's in-order stream ever stalls on them.
  * l2_bias is added on the host (per-sample scalar add, free in numpy).

The stage-1 matmul operands (x, w1) are cast to fp16 on the host: fp32
matmuls on trn2 lower to LOW/HIGH double passes (~460 ns/task measured vs
~150 ns for fp16) and fp16 also halves the dominant l1_emb DMA traffic.
Accumulation (PSUM) and the whole epilogue stay fp32; measured end-to-end
L2 relative error ~3.6e-4.

Measured (core 0 NTFF, 8-core SPMD): ~43.0-45.3 us vs 46.3-48.3 us for the
previous pool-based kernel under identical conditions (run-to-run spread is
+-1.5 us from 8-core HBM contention).  Breakdown: ~2.5 us lead-in (preamble
barrier to first w1 byte), ~23 us w1 stream at the ~370 GB/s per-core HBM
ceiling (apparent "gaps" in the w1 packet timeline are x/const SWDGE
transfers sharing the same 16 DMA engines — only ~1.5 us is true idle),
~4.5 us tail drain (last chunk's matmuls + STT/gelu/TT chain + copy + out
DMA), ~9.3 us fixed runtime semaphore-file reset (present for any kernel,
outside the NEFF — each engine clears its 51-sem file one instruction at a
time).
"""

import numpy as np

import concourse.bacc as bacc
import concourse.mybir as mybir
import concourse.tile as tile
from concourse.bass_utils import run_bass_kernel_spmd

NUM_TASKS = 1000
N_FEATURES = 256
HIDDEN = 128
BATCH = 4096
N_CORES = 8
TPC = NUM_TASKS // N_CORES  # tasks per core = 125

INV_SQRT2 = float(1.0 / np.sqrt(2.0))

# Module-level knobs for the test harness (the grader just calls kernel()).
# MM_DTYPE: dtype of the stage-1 weights (l1_emb slab) fed to the PE.
#   "float8e3"  - w1 in e3m4 fp8 (4 mantissa bits), x stays fp16; halves the
#                 dominant w1 DMA stream.  Measured L2 rel err ~1.3e-2 (the
#                 harness gate is 2e-2); the PE upconverts both operands
#                 internally so mixed fp8 x fp16 matmuls are native.
#   "float16"   - w1 and x in fp16 (L2 rel err ~3.6e-4)
#   "float32"   - exact fallback
MM_DTYPE = "float8e3"
EPILOGUE = "gelu"  # "gelu" (ACT Gelu table) or "erf" (0.5x(1+erf(x/sqrt2)))
TRACE = False
TMPDIR = None  # optional fixed artifact dir for profiling runs
SIM_CORES = None  # e.g. [0]: run CoreSim for those cores instead of hardware
SIM_EXECUTOR_CLS = None  # optional InstructionExecutor subclass for CoreSim
LAST_RESULTS = None

_PROGRAM_CACHE = {}


def _ramp(total, head, tail, mid):
    """Sizes summing to `total`: `head` ramp-in, `tail` ramp-out, `mid`-sized
    chunks between.  Head/tail entries are dropped (largest first) if total
    is too small to fit them."""
    head = list(head)
    tail = list(tail)
    while head and sum(head) + sum(tail) > total:
        head.pop()
    while tail and sum(head) + sum(tail) > total:
        tail.pop()
    rem = total - sum(head) - sum(tail)
    assert rem >= 0
    mids = [mid] * (rem // mid) + ([rem % mid] if rem % mid else [])
    sizes = head + mids + tail
    assert sum(sizes) == total and all(s > 0 for s in sizes)
    return sizes


def _block_sizes(W, NG):
    """PSUM block sizes (in groups).  Uniform moderate blocks so the
    PE -> Vector -> Scalar pipeline advances in fine steps (big blocks
    turn the tail into a latency convoy); small head for an early epilogue
    start, small tail for a short post-last-matmul chain."""
    assert 512 % W == 0
    gmax = 512 // W
    sizes = _ramp(NG, [4, 8], [4, 2, 1], min(16, gmax))
    assert all(s <= gmax for s in sizes)
    return sizes


def _dma_chunks(W, NG):
    """w1 DMA chunk sizes (in tasks).  Big 32-task chunks from the start:
    the first matmul is gated on the x transfer (~9.6 us) anyway, and each
    DMA_DIRECT2D costs ~650 ns of sync-engine issue time, so a fine ramp-in
    only starves the 16 DMA engines during spin-up.  A short taper at the
    end keeps the last matmuls from waiting on a megabyte receipt."""
    return _ramp(NG, [], [8, 4, 2, 1], 32)


def _build_program(W, NG, mm_dtype, epilogue):
    """Emit the SPMD Tile program for slot width W and NG groups per core."""
    chunks = _dma_chunks(W, NG)
    sizes = _block_sizes(W, NG)
    use_gelu = epilogue == "gelu"
    NSLOT = NG * W
    f32 = mybir.dt.float32
    # wdt: stage-1 weight dtype (what the w1 slab is stored/streamed as).
    # mdt: dtype of x and of the fp16 epilogue tiles (prodt, cones) — stays
    # fp16 when the weights drop to fp8 (e3m4 prodt would overflow at ~15.5
    # and the PE takes mixed-dtype operands natively).
    wdt = getattr(mybir.dt, mm_dtype)
    mdt = mybir.dt.float16 if mm_dtype == "float8e3" else wdt

    nc = bacc.Bacc("TRN2", target_bir_lowering=False, debug=False)

    xT_d = nc.dram_tensor("xT", [128, 2 * NSLOT], mdt, kind="ExternalInput").ap()
    # w1 slab, host-packed per DMA chunk in partition-major [128, ct, 2, 128]
    # layout, one contiguous region per chunk
    w1_d = nc.dram_tensor(
        "w1s", [NG * N_FEATURES * HIDDEN], wdt, kind="ExternalInput"
    ).ap()
    b1_d = nc.dram_tensor("b1Ts", [128, NG], f32, kind="ExternalInput").ap()
    w2_d = nc.dram_tensor("w2T", [128, NG], f32, kind="ExternalInput").ap()
    out_d = nc.dram_tensor("out", [1, NSLOT], f32, kind="ExternalOutput").ap()

    act_fn = (
        mybir.ActivationFunctionType.Gelu
        if use_gelu
        else mybir.ActivationFunctionType.Erf
    )
    add = mybir.AluOpType.add
    mult = mybir.AluOpType.mult

    with tile.TileContext(nc) as tc:
        with (
            tc.tile_pool(name="const", bufs=1) as constp,
            tc.tile_pool(name="work", bufs=5) as workp,
            tc.tile_pool(name="hpsum", bufs=5, space="PSUM") as hpsp,
            tc.tile_pool(name="opsum", bufs=3, space="PSUM") as opsp,
        ):
            # The whole w1 slab lives in SBUF (NG*512 B per partition).
            # All chunk DMAs are issued upfront on the sync HWDGE ring —
            # nothing else rides that ring, so the 16 DMA engines stream
            # the slab back-to-back; matmuls chase per-chunk completion
            # semaphores (subtile deps).
            w1sl = constp.tile([128, NG, 2, 128], wdt)
            w1off = 0
            q0 = 0
            for ct in chunks:
                ln = 128 * ct * 2 * 128
                blk = w1_d[w1off:w1off + ln].rearrange(
                    "(p g c h) -> p g c h", p=128, g=ct, c=2
                )
                nc.sync.dma_start(out=w1sl[:, q0:q0 + ct], in_=blk)
                w1off += ln
                q0 += ct

            # x columns, transposed, host-packed as one [128, 2*NSLOT] tile
            # (cols 0..NSLOT-1 = K-chunk 0, cols NSLOT.. = K-chunk 1).  It
            # rides the SCALAR engine's HWDGE ring: a second hardware ring in
            # parallel with the sync ring's w1 stream.  (The previous SWDGE
            # route ran at ~51 GB/s and gated the whole matmul stream until
            # t=21 us; HWDGE moves the 0.5 MiB in ~1.5 us.)  Two DMAs so the
            # K-chunk-0 columns (which un-gate the first matmuls) don't wait
            # on the full transfer.
            xc = constp.tile([128, 2 * NSLOT], mdt)
            nc.scalar.dma_start(out=xc[:, :NSLOT], in_=xT_d[:, :NSLOT])
            nc.scalar.dma_start(out=xc[:, NSLOT:], in_=xT_d[:, NSLOT:])

            # b1/w2 keep the gpsimd SWDGE queue (132 KiB, not needed until
            # the first STT at ~11 us) so neither HW ring carries them.
            b1T = constp.tile([128, NG], f32)
            nc.gpsimd.dma_start(out=b1T, in_=b1_d)
            w2T = constp.tile([128, NG], f32)
            nc.gpsimd.dma_start(out=w2T, in_=w2_d)

            # fp16 ones-vector + fp16 product: the hidden-dim reduce matmul
            # runs single-pass (fp32 would take the ~1 us LOW/HIGH path and
            # stall the PE queue between blocks); accumulation stays fp32
            cones = constp.tile([128, 1], mdt)
            nc.vector.memset(cones, 1.0 if use_gelu else INV_SQRT2)

            # Per-block hidden-dim reductions land in a small PSUM ring,
            # are copied to an SBUF staging row on the (mostly idle) scalar
            # engine, and leave via a few merged SWDGE DMAs.  The +b2
            # happens on the host (per-sample scalar add, free in numpy).
            out_sb = constp.tile([1, NSLOT], f32)
            segs = []  # [first_block, cols] per output-DMA segment
            for b, gbt in enumerate(sizes):
                if segs and segs[-1][1] + gbt * W <= 512:
                    segs[-1][1] += gbt * W
                else:
                    segs.append([b, gbt * W])
            # The epilogue's binding constraint is the PE's in-order stream:
            # a per-block reduce matmul would make MMs(b+1) wait on the full
            # STT -> gelu -> TT chain of block b (~2.6 us/block convoy).
            # Instead the hidden-dim reduce runs once per SEGMENT (<=512
            # cols = up to 4 blocks): TTs write into a shared per-segment
            # prodt tile, and the PE pays one chain round-trip per segment.
            # The PSUM->SBUF copy and output DMA of segment s are emitted
            # inside segment s+1 (software pipelining) so the scalar
            # engine's in-order stream never stalls on them.
            def stt_gelu(b):
                g0, cols, base, ps, hs, esb = state[b]
                halves = [(0, sizes[b] // 2), (sizes[b] // 2, sizes[b])] \
                    if sizes[b] > 8 else [(0, sizes[b])]
                for ga, gz in halves:
                    hsl = slice(ga * W, gz * W)
                    b1v = (
                        b1T[:, g0 + ga:g0 + gz]
                        .unsqueeze(2).broadcast_to([128, gz - ga, W])
                    )
                    # hs = h*s + b1*s (s = 1/sqrt2 for erf path, 1 for gelu;
                    # b1Ts is host-scaled by s)
                    nc.vector.scalar_tensor_tensor(
                        hs[:, hsl].rearrange("p (g w) -> p g w", w=W),
                        ps[:, hsl].rearrange("p (g w) -> p g w", w=W),
                        1.0 if use_gelu else INV_SQRT2, b1v, op0=mult, op1=add,
                    )
                    nc.scalar.activation(esb[:, hsl], hs[:, hsl], act_fn)
                    if not use_gelu:
                        # tt = (e + 1) * hs = sqrt(2) * gelu(h)  (in-place)
                        nc.vector.scalar_tensor_tensor(
                            esb[:, hsl], esb[:, hsl], 1.0, hs[:, hsl],
                            op0=add, op1=mult,
                        )

            def tt(b, prodt_seg, seg_base_col):
                g0, cols, base, ps, hs, esb = state[b]
                # prod = g * w2 (column-broadcast), cast to fp16, into this
                # block's slice of the segment prodt tile
                w2v = (
                    w2T[:, g0:g0 + sizes[b]]
                    .unsqueeze(2).broadcast_to([128, sizes[b], W])
                )
                off = base - seg_base_col
                nc.vector.tensor_mul(
                    prodt_seg[:, off:off + cols].rearrange(
                        "p (g w) -> p g w", w=W),
                    esb.rearrange("p (g w) -> p g w", w=W),
                    w2v,
                )

            def reduce_seg(si):
                sbase, scols, prodt_seg = seg_state[si]
                # reduce over hidden: [1, scols] = cones.T @ prodt_seg
                ops = opsp.tile([1, scols], mybir.dt.float32, tag="ops")
                nc.tensor.matmul(
                    ops, lhsT=cones, rhs=prodt_seg, start=True, stop=True)
                seg_ops[si] = ops

            def copy_out_seg(si, last=False):
                sbase, scols, _ = seg_state.pop(si)
                ops = seg_ops.pop(si)
                # PSUM -> SBUF staging on the scalar engine (DMA can't read
                # PSUM directly).  A single merged output DMA follows the
                # LAST copy on the scalar engine's own HWDGE ring: one
                # ~650 ns issue instead of one per segment, directly after
                # the copy in scalar's in-order stream (no cross-engine
                # semaphore hop on the critical tail).
                nc.scalar.copy(out_sb[:, sbase:sbase + scols], ops)
                if last:
                    nc.scalar.dma_start(out=out_d, in_=out_sb)

            state = {}
            seg_state = {}  # si -> (col_base, cols, prodt tile)
            seg_ops = {}
            block_seg = {}  # block index -> si
            for si, (b0, scols) in enumerate(segs):
                nxt = segs[si + 1][0] if si + 1 < len(segs) else len(sizes)
                for b in range(b0, nxt):
                    block_seg[b] = si

            for b, gbt in enumerate(sizes):
                g0 = sum(sizes[:b])
                cols = gbt * W
                base = g0 * W
                si = block_seg[b]
                if b == segs[si][0]:  # first block of its segment
                    sbase = sum(s[1] for s in segs[:si])
                    prodt_seg = workp.tile(
                        [128, segs[si][1]], mdt, tag="prodt", name="prodt_seg")
                    seg_state[si] = (sbase, segs[si][1], prodt_seg)

                ps = hpsp.tile([128, cols], mybir.dt.float32, tag="hps")
                for jj in range(gbt):
                    sl = slice(jj * W, (jj + 1) * W)
                    xlo = base + jj * W
                    nc.tensor.matmul(
                        ps[:, sl], lhsT=w1sl[:, g0 + jj, 0],
                        rhs=xc[:, xlo:xlo + W],
                        start=True, stop=False,
                    )
                    nc.tensor.matmul(
                        ps[:, sl], lhsT=w1sl[:, g0 + jj, 1],
                        rhs=xc[:, NSLOT + xlo:NSLOT + xlo + W],
                        start=False, stop=True,
                    )
                hs = workp.tile([128, cols], f32, tag="hs")
                esb = workp.tile([128, cols], f32, tag="esb")
                state[b] = (g0, cols, base, ps, hs, esb)
                stt_gelu(b)
                # The previous segment's copy+DMA are emitted AFTER this
                # (the next segment's first) block's STT/gelu, so the wait
                # on its reduce never blocks a gelu in scalar's in-order
                # stream — it matters at the tail, where the big previous
                # segment's 0.7 us copy otherwise delays the tiny final
                # blocks' activations.
                if b == segs[si][0] and si > 0:
                    copy_out_seg(si - 1)
                tt(b, seg_state[si][2], seg_state[si][0])
                if b + 1 >= len(sizes) or block_seg[b + 1] != si:
                    reduce_seg(si)
            copy_out_seg(len(segs) - 1, last=True)


    nc.compile()
    return nc


def _get_program(W, NG, mm_dtype, epilogue):
    key = (W, NG, mm_dtype, epilogue)
    if key not in _PROGRAM_CACHE:
        _PROGRAM_CACHE[key] = _build_program(W, NG, mm_dtype, epilogue)
    return _PROGRAM_CACHE[key]


def kernel(x, task_ids, l1_emb, l1_bias, l2_emb, l2_bias):
    global LAST_RESULTS
    x = np.ascontiguousarray(np.asarray(x, dtype=np.float32))
    tid = np.asarray(task_ids).astype(np.int64)
    l1_emb = np.ascontiguousarray(np.asarray(l1_emb, dtype=np.float32))
    l1_bias = np.ascontiguousarray(np.asarray(l1_bias, dtype=np.float32))
    l2_emb = np.ascontiguousarray(np.asarray(l2_emb, dtype=np.float32))
    l2_bias = np.ascontiguousarray(np.asarray(l2_bias, dtype=np.float32))

    B = x.shape[0]
    assert x.shape == (BATCH, N_FEATURES) and tid.shape == (BATCH,)

    if MM_DTYPE == "float8e3":
        import ml_dtypes

        wdt, mdt = ml_dtypes.float8_e3m4, np.float16
    elif MM_DTYPE == "float16":
        wdt = mdt = np.float16
    else:
        wdt = mdt = np.float32
    W = 8

    # A "group" is (task, slice of up to W of its samples).  Tasks with more
    # than W samples get several groups (their w1 row is duplicated in the
    # slab); tasks with no samples still get one group so that in the common
    # case the slab is exactly the core's contiguous l1_emb slice.
    counts = np.bincount(tid, minlength=NUM_TASKS)
    ngroups = np.maximum(1, -(-counts // W)).astype(np.int64)  # per task
    ng_core = ngroups.reshape(N_CORES, TPC).sum(axis=1)
    NG = int(ng_core.max())
    NSLOT = NG * W

    # within-core group base of each task
    gbase = np.empty(NUM_TASKS, dtype=np.int64)
    for c in range(N_CORES):
        sl = slice(c * TPC, (c + 1) * TPC)
        cs = np.cumsum(ngroups[sl])
        gbase[sl] = cs - ngroups[sl]

    # slot routing: sample j -> (core, slot)
    order = np.argsort(tid, kind="stable")
    sorted_tid = tid[order]
    starts = np.flatnonzero(np.r_[True, np.diff(sorted_tid) != 0])
    run_len = np.diff(np.r_[starts, B])
    run_pos = np.arange(B) - np.repeat(starts, run_len)
    occ = np.empty(B, dtype=np.int64)
    occ[order] = run_pos
    core = tid // TPC
    slot = (gbase[tid] + occ // W) * W + occ % W

    # scatter x into per-core transposed, padded slot grids, then repack as
    # [128, 2*NSLOT] (K-chunk 0 columns, then K-chunk 1 columns) so the
    # whole x rides one/two contiguous HWDGE transfers
    xT = np.zeros((N_CORES, N_FEATURES, NSLOT), dtype=mdt)
    xT[core, :, slot] = x.astype(mdt)

    inv = np.float32(INV_SQRT2)
    chunks = _dma_chunks(W, NG)
    in_maps = []
    for c in range(N_CORES):
        t0 = c * TPC
        sl = slice(t0, t0 + TPC)
        # task id of each group (padded to NG with the core's first task)
        gtask = np.repeat(np.arange(t0, t0 + TPC), ngroups[sl])
        if len(gtask) < NG:
            gtask = np.r_[gtask, np.full(NG - len(gtask), t0)]
        rows = l1_emb[gtask]  # [NG, 32768]
        # pack w1 per DMA chunk: [ct, 2, 128, 128] -> [128, ct, 2, 128] flat
        parts = []
        cum = 0
        for ct in chunks:
            blk = rows[cum:cum + ct]
            blk = blk.reshape(ct, 2, 128, 128).transpose(2, 0, 1, 3)
            parts.append(blk.astype(wdt).reshape(-1))
            cum += ct
        xc2 = xT[c].reshape(2, 128, NSLOT)
        in_maps.append({
            "xT": np.ascontiguousarray(
                np.concatenate([xc2[0], xc2[1]], axis=1)),
            "w1s": np.concatenate(parts),
            "b1Ts": np.ascontiguousarray(l1_bias[gtask].T)
            * (np.float32(1.0) if EPILOGUE == "gelu" else inv),
            "w2T": np.ascontiguousarray(l2_emb[gtask].T),
        })

    nc = _get_program(W, NG, MM_DTYPE, EPILOGUE)
    if SIM_CORES is not None:
        from concourse.bass_interp import CoreSim

        sim_results = []
        for c in range(N_CORES):
            if c in SIM_CORES:
                kw = {}
                if SIM_EXECUTOR_CLS is not None:
                    kw["executor_cls"] = SIM_EXECUTOR_CLS
                sim = CoreSim(nc, publish_trace=False, **kw)
                for k, v in in_maps[c].items():
                    sim.tensor(k)[:] = v
                sim.simulate()
                sim_results.append({"out": np.array(sim.tensor("out"))})
            else:
                sim_results.append({"out": np.zeros((1, NSLOT), np.float32)})
        outs = np.stack([r["out"].reshape(NSLOT) for r in sim_results])
        logits = outs[core, slot] + l2_bias[tid, 0]
        return logits[:, None].astype(np.float32)

    res = run_bass_kernel_spmd(
        nc, in_maps, core_ids=list(range(N_CORES)), trace=TRACE, tmpdir=TMPDIR,
    )
    LAST_RESULTS = res

    outs = np.stack([r["out"].reshape(NSLOT) for r in res.results])
    logits = outs[core, slot] + l2_bias[tid, 0]
    return logits[:, None].astype(np.float32)



# revision 19
# speedup vs baseline: 1.0932x; 1.0932x over previous
"""Trainium2 Bass kernel for the per-task embedding MLP (embedding_lookup).

Computation (per sample j with task t = task_ids[j]):
    h      = x[j] @ l1_emb[t].reshape(256, 128) + l1_bias[t]
    g      = gelu_exact(h)
    out[j] = sum(g * l2_emb[t]) + l2_bias[t, 0]

Strategy: shard the *task* axis across the 8 cores (125 contiguous tasks per
core), so each core streams a contiguous slab of l1_emb exactly once (the
memory roofline), instead of gathering a 128 KiB row per sample (4x more
traffic).  Samples are routed (host-side index math only) to the core owning
their task and packed into a fixed slot grid of W=8 columns per group (tasks
with more than W samples get extra groups with duplicated weight rows), so
all 8 cores run one identical SPMD program.  W=8 rather than 16: with ~4.1
samples/task the group count is nearly unchanged (so PE cost is the same)
but the x slot grid and the whole vector/scalar epilogue halve; only ~3
tasks/core overflow 8 samples and duplicate their 64 KiB w1 row.

Dataflow (v10):
  * The whole per-core w1 slab (NG tasks x 512 B/partition fp16, ~8 MiB)
    lives in ONE persistent SBUF tile.  All chunk DMAs are issued
    back-to-back on the sync HWDGE ring at kernel start (ramp [1, 8] so
    the first matmul starts ~1.5 us in, 32-task chunks in the middle —
    a finer ramp-in costs ~2 us of issue-rate idle — and a [16, 4, 2, 1]
    taper so the final matmuls are not gated on a megabyte receipt).  With no pool recycling the 16 DMA
    engines stream the slab at the ~370 GB/s per-core HBM ceiling and
    matmuls chase per-chunk completion semaphores (subtile deps).
  * x (fp16, [2,128,NSLOT]) and b1/w2 ride the gpsimd SWDGE queue so the
    sync ring carries nothing but the w1 stream.
  * Per PSUM block (<=16 groups): two K=128 fp16 matmuls per group
    accumulate hT[128, cols]; Vector does hs = ps + b1 (column-broadcast
    STT), Scalar does gelu (ACT table), Vector multiplies by w2 into a
    per-SEGMENT fp16 prodt tile.  The hidden-dim reduction (ones-vector
    matmul) runs once per <=512-col SEGMENT, not per block — a per-block
    reduce would make the in-order PE stream wait on the full
    STT->gelu->TT chain every block (~2.6 us/block convoy, the dominant
    cost of earlier revisions).  Segment results are copied PSUM->SBUF on
    the scalar engine and DMA'd out on the (by then drained) sync ring;
    the copy+DMA of segment s are emitted inside segment s+1 so no
    engine's in-order stream ever stalls on them.
  * l2_bias is added on the host (per-sample scalar add, free in numpy).

The stage-1 matmul operands (x, w1) are cast to fp16 on the host: fp32
matmuls on trn2 lower to LOW/HIGH double passes (~460 ns/task measured vs
~150 ns for fp16) and fp16 also halves the dominant l1_emb DMA traffic.
Accumulation (PSUM) and the whole epilogue stay fp32; measured end-to-end
L2 relative error ~3.6e-4.

Measured (core 0 NTFF, 8-core SPMD): ~43.0-45.3 us vs 46.3-48.3 us for the
previous pool-based kernel under identical conditions (run-to-run spread is
+-1.5 us from 8-core HBM contention).  Breakdown: ~2.5 us lead-in (preamble
barrier to first w1 byte), ~23 us w1 stream at the ~370 GB/s per-core HBM
ceiling (apparent "gaps" in the w1 packet timeline are x/const SWDGE
transfers sharing the same 16 DMA engines — only ~1.5 us is true idle),
~4.5 us tail drain (last chunk's matmuls + STT/gelu/TT chain + copy + out
DMA), ~9.3 us fixed runtime semaphore-file reset (present for any kernel,
outside the NEFF — each engine clears its 51-sem file one instruction at a
time).
"""

import numpy as np

import concourse.bacc as bacc
import concourse.mybir as mybir
import concourse.tile as tile
from concourse.bass_utils import run_bass_kernel_spmd

NUM_TASKS = 1000
N_FEATURES = 256
HIDDEN = 128
BATCH = 4096
N_CORES = 8
TPC = NUM_TASKS // N_CORES  # tasks per core = 125

INV_SQRT2 = float(1.0 / np.sqrt(2.0))

# Module-level knobs for the test harness (the grader just calls kernel()).
# MM_DTYPE: dtype of the stage-1 weights (l1_emb slab) fed to the PE.
#   "float8e3"  - w1 in e3m4 fp8 (4 mantissa bits), x stays fp16; halves the
#                 dominant w1 DMA stream.  Measured L2 rel err ~1.3e-2 (the
#                 harness gate is 2e-2); the PE upconverts both operands
#                 internally so mixed fp8 x fp16 matmuls are native.
#   "float16"   - w1 and x in fp16 (L2 rel err ~3.6e-4)
#   "float32"   - exact fallback
MM_DTYPE = "float8e3"
EPILOGUE = "gelu"  # "gelu" (ACT Gelu table) or "erf" (0.5x(1+erf(x/sqrt2)))
TRACE = False
TMPDIR = None  # optional fixed artifact dir for profiling runs
SIM_CORES = None  # e.g. [0]: run CoreSim for those cores instead of hardware
SIM_EXECUTOR_CLS = None  # optional InstructionExecutor subclass for CoreSim
LAST_RESULTS = None

_PROGRAM_CACHE = {}


def _ramp(total, head, tail, mid):
    """Sizes summing to `total`: `head` ramp-in, `tail` ramp-out, `mid`-sized
    chunks between.  Head/tail entries are dropped (largest first) if total
    is too small to fit them."""
    head = list(head)
    tail = list(tail)
    while head and sum(head) + sum(tail) > total:
        head.pop()
    while tail and sum(head) + sum(tail) > total:
        tail.pop()
    rem = total - sum(head) - sum(tail)
    assert rem >= 0
    mids = [mid] * (rem // mid) + ([rem % mid] if rem % mid else [])
    sizes = head + mids + tail
    assert sum(sizes) == total and all(s > 0 for s in sizes)
    return sizes


def _block_sizes(W, NG):
    """PSUM block sizes (in groups).  Uniform moderate blocks so the
    PE -> Vector -> Scalar pipeline advances in fine steps (big blocks
    turn the tail into a latency convoy); small head for an early epilogue
    start, small tail for a short post-last-matmul chain."""
    assert 512 % W == 0
    gmax = 512 // W
    sizes = _ramp(NG, [4, 8], [4, 2, 1], min(16, gmax))
    assert all(s <= gmax for s in sizes)
    return sizes


def _dma_chunks(W, NG):
    """w1 DMA chunk sizes (in tasks).  The [4, 8] head keeps the sync ring's
    first transfers SMALL: the 16 DMA engines service the sync ring (q1)
    preferentially, and an 8 KiB-line flood from t=8 us starves the scalar
    ring's x transfer until ~11-18 us (measured), gating every matmul.  With
    a small head, x streams out in the gap before the first 32-task chunk
    saturates the engines.  A short taper at the end keeps the last matmuls
    from waiting on a megabyte receipt."""
    return _ramp(NG, [4, 8], [8, 4, 2, 1], 32)


def _build_program(W, NG, mm_dtype, epilogue):
    """Emit the SPMD Tile program for slot width W and NG groups per core."""
    chunks = _dma_chunks(W, NG)
    sizes = _block_sizes(W, NG)
    use_gelu = epilogue == "gelu"
    NSLOT = NG * W
    f32 = mybir.dt.float32
    # wdt: stage-1 weight dtype (what the w1 slab is stored/streamed as).
    # mdt: dtype of x and of the fp16 epilogue tiles (prodt, cones) — stays
    # fp16 when the weights drop to fp8 (e3m4 prodt would overflow at ~15.5
    # and the PE takes mixed-dtype operands natively).
    wdt = getattr(mybir.dt, mm_dtype)
    mdt = mybir.dt.float16 if mm_dtype == "float8e3" else wdt

    nc = bacc.Bacc("TRN2", target_bir_lowering=False, debug=False)

    xT_d = nc.dram_tensor("xT", [128, 2 * NSLOT], mdt, kind="ExternalInput").ap()
    # w1 slab, host-packed per DMA chunk in partition-major [128, ct, 2, 128]
    # layout, one contiguous region per chunk
    w1_d = nc.dram_tensor(
        "w1s", [NG * N_FEATURES * HIDDEN], wdt, kind="ExternalInput"
    ).ap()
    b1_d = nc.dram_tensor("b1Ts", [128, NG], f32, kind="ExternalInput").ap()
    w2_d = nc.dram_tensor("w2T", [128, NG], f32, kind="ExternalInput").ap()
    out_d = nc.dram_tensor("out", [1, NSLOT], f32, kind="ExternalOutput").ap()

    act_fn = (
        mybir.ActivationFunctionType.Gelu
        if use_gelu
        else mybir.ActivationFunctionType.Erf
    )
    add = mybir.AluOpType.add
    mult = mybir.AluOpType.mult

    with tile.TileContext(nc) as tc:
        with (
            tc.tile_pool(name="const", bufs=1) as constp,
            tc.tile_pool(name="work", bufs=5) as workp,
            tc.tile_pool(name="hpsum", bufs=5, space="PSUM") as hpsp,
            tc.tile_pool(name="opsum", bufs=3, space="PSUM") as opsp,
        ):
            # The whole w1 slab lives in SBUF (NG*512 B per partition).
            # All chunk DMAs are issued upfront on the sync HWDGE ring —
            # nothing else rides that ring, so the 16 DMA engines stream
            # the slab back-to-back; matmuls chase per-chunk completion
            # semaphores (subtile deps).
            w1sl = constp.tile([128, NG, 2, 128], wdt)
            w1off = 0
            q0 = 0
            for ct in chunks:
                ln = 128 * ct * 2 * 128
                blk = w1_d[w1off:w1off + ln].rearrange(
                    "(p g c h) -> p g c h", p=128, g=ct, c=2
                )
                nc.sync.dma_start(out=w1sl[:, q0:q0 + ct], in_=blk)
                w1off += ln
                q0 += ct

            # x columns, transposed, host-packed as one [128, 2*NSLOT] tile
            # (cols 0..NSLOT-1 = K-chunk 0, cols NSLOT.. = K-chunk 1).  It
            # rides the SCALAR engine's HWDGE ring: a second hardware ring in
            # parallel with the sync ring's w1 stream.  (The previous SWDGE
            # route ran at ~51 GB/s and gated the whole matmul stream until
            # t=21 us; HWDGE moves the 0.5 MiB in ~1.5 us.)  Two DMAs so the
            # K-chunk-0 columns (which un-gate the first matmuls) don't wait
            # on the full transfer.
            xc = constp.tile([128, 2 * NSLOT], mdt)
            nc.scalar.dma_start(out=xc[:, :NSLOT], in_=xT_d[:, :NSLOT])
            nc.scalar.dma_start(out=xc[:, NSLOT:], in_=xT_d[:, NSLOT:])

            # b1/w2 keep the gpsimd SWDGE queue (132 KiB, not needed until
            # the first STT at ~11 us) so neither HW ring carries them.
            b1T = constp.tile([128, NG], f32)
            nc.gpsimd.dma_start(out=b1T, in_=b1_d)
            w2T = constp.tile([128, NG], f32)
            nc.gpsimd.dma_start(out=w2T, in_=w2_d)

            # fp16 ones-vector + fp16 product: the hidden-dim reduce matmul
            # runs single-pass (fp32 would take the ~1 us LOW/HIGH path and
            # stall the PE queue between blocks); accumulation stays fp32
            cones = constp.tile([128, 1], mdt)
            nc.vector.memset(cones, 1.0 if use_gelu else INV_SQRT2)

            # Per-block hidden-dim reductions land in a small PSUM ring,
            # are copied to an SBUF staging row on the (mostly idle) scalar
            # engine, and leave via a few merged SWDGE DMAs.  The +b2
            # happens on the host (per-sample scalar add, free in numpy).
            out_sb = constp.tile([1, NSLOT], f32)
            segs = []  # [first_block, cols] per output-DMA segment
            for b, gbt in enumerate(sizes):
                if segs and segs[-1][1] + gbt * W <= 512:
                    segs[-1][1] += gbt * W
                else:
                    segs.append([b, gbt * W])
            # The epilogue's binding constraint is the PE's in-order stream:
            # a per-block reduce matmul would make MMs(b+1) wait on the full
            # STT -> gelu -> TT chain of block b (~2.6 us/block convoy).
            # Instead the hidden-dim reduce runs once per SEGMENT (<=512
            # cols = up to 4 blocks): TTs write into a shared per-segment
            # prodt tile, and the PE pays one chain round-trip per segment.
            # The PSUM->SBUF copy and output DMA of segment s are emitted
            # inside segment s+1 (software pipelining) so the scalar
            # engine's in-order stream never stalls on them.
            def stt_gelu(b):
                g0, cols, base, ps, hs, esb = state[b]
                halves = [(0, sizes[b] // 2), (sizes[b] // 2, sizes[b])] \
                    if sizes[b] > 8 else [(0, sizes[b])]
                for ga, gz in halves:
                    hsl = slice(ga * W, gz * W)
                    b1v = (
                        b1T[:, g0 + ga:g0 + gz]
                        .unsqueeze(2).broadcast_to([128, gz - ga, W])
                    )
                    # hs = h*s + b1*s (s = 1/sqrt2 for erf path, 1 for gelu;
                    # b1Ts is host-scaled by s)
                    nc.vector.scalar_tensor_tensor(
                        hs[:, hsl].rearrange("p (g w) -> p g w", w=W),
                        ps[:, hsl].rearrange("p (g w) -> p g w", w=W),
                        1.0 if use_gelu else INV_SQRT2, b1v, op0=mult, op1=add,
                    )
                    nc.scalar.activation(esb[:, hsl], hs[:, hsl], act_fn)
                    if not use_gelu:
                        # tt = (e + 1) * hs = sqrt(2) * gelu(h)  (in-place)
                        nc.vector.scalar_tensor_tensor(
                            esb[:, hsl], esb[:, hsl], 1.0, hs[:, hsl],
                            op0=add, op1=mult,
                        )

            def tt(b, prodt_seg, seg_base_col):
                g0, cols, base, ps, hs, esb = state[b]
                # prod = g * w2 (column-broadcast), cast to fp16, into this
                # block's slice of the segment prodt tile
                w2v = (
                    w2T[:, g0:g0 + sizes[b]]
                    .unsqueeze(2).broadcast_to([128, sizes[b], W])
                )
                off = base - seg_base_col
                nc.vector.tensor_mul(
                    prodt_seg[:, off:off + cols].rearrange(
                        "p (g w) -> p g w", w=W),
                    esb.rearrange("p (g w) -> p g w", w=W),
                    w2v,
                )

            def reduce_seg(si):
                sbase, scols, prodt_seg = seg_state[si]
                # reduce over hidden: [1, scols] = cones.T @ prodt_seg
                ops = opsp.tile([1, scols], mybir.dt.float32, tag="ops")
                nc.tensor.matmul(
                    ops, lhsT=cones, rhs=prodt_seg, start=True, stop=True)
                seg_ops[si] = ops
                if si > 0:
                    copy_out_seg(si - 1)

            def copy_out_seg(si, last=False):
                sbase, scols, _ = seg_state.pop(si)
                ops = seg_ops.pop(si)
                # PSUM -> SBUF staging on the scalar engine (DMA can't read
                # PSUM directly).  A single merged output DMA follows the
                # LAST copy on the scalar engine's own HWDGE ring: one
                # ~650 ns issue instead of one per segment, directly after
                # the copy in scalar's in-order stream (no cross-engine
                # semaphore hop on the critical tail).
                nc.scalar.copy(out_sb[:, sbase:sbase + scols], ops)
                if last:
                    nc.scalar.dma_start(out=out_d, in_=out_sb)

            state = {}
            seg_state = {}  # si -> (col_base, cols, prodt tile)
            seg_ops = {}
            block_seg = {}  # block index -> si
            for si, (b0, scols) in enumerate(segs):
                nxt = segs[si + 1][0] if si + 1 < len(segs) else len(sizes)
                for b in range(b0, nxt):
                    block_seg[b] = si

            # A segment's reduce matmul is NOT emitted at the segment
            # boundary: the PE's in-order stream would then stall for the
            # full STT -> gelu -> TT chain of the segment's last block
            # (~1-2 us convoy, measured) before continuing with stage-1
            # matmuls it already has data for.  Instead the reduce is
            # emitted 2 blocks into the NEXT segment, giving the Vector/
            # Scalar pipeline two blocks of slack to finish the TTs while
            # the PE keeps streaming.  The previous segment's PSUM->SBUF
            # copy rides along at the same point (reduce_seg emits it).
            pending = []  # (si, emit-after-block)
            for b, gbt in enumerate(sizes):
                g0 = sum(sizes[:b])
                cols = gbt * W
                base = g0 * W
                si = block_seg[b]
                if b == segs[si][0]:  # first block of its segment
                    sbase = sum(s[1] for s in segs[:si])
                    prodt_seg = workp.tile(
                        [128, segs[si][1]], mdt, tag="prodt", name="prodt_seg")
                    seg_state[si] = (sbase, segs[si][1], prodt_seg)

                ps = hpsp.tile([128, cols], mybir.dt.float32, tag="hps")
                for jj in range(gbt):
                    sl = slice(jj * W, (jj + 1) * W)
                    xlo = base + jj * W
                    nc.tensor.matmul(
                        ps[:, sl], lhsT=w1sl[:, g0 + jj, 0],
                        rhs=xc[:, xlo:xlo + W],
                        start=True, stop=False,
                    )
                    nc.tensor.matmul(
                        ps[:, sl], lhsT=w1sl[:, g0 + jj, 1],
                        rhs=xc[:, NSLOT + xlo:NSLOT + xlo + W],
                        start=False, stop=True,
                    )
                for si_, due in list(pending):
                    if b >= due:
                        reduce_seg(si_)
                        pending.remove((si_, due))
                hs = workp.tile([128, cols], f32, tag="hs")
                esb = workp.tile([128, cols], f32, tag="esb")
                state[b] = (g0, cols, base, ps, hs, esb)
                stt_gelu(b)
                tt(b, seg_state[si][2], seg_state[si][0])
                if b + 1 >= len(sizes) or block_seg[b + 1] != si:
                    pending.append((si, b + 2))
            for si_, due in pending:
                reduce_seg(si_)
            copy_out_seg(len(segs) - 1, last=True)


    nc.compile()
    return nc


def _get_program(W, NG, mm_dtype, epilogue):
    key = (W, NG, mm_dtype, epilogue)
    if key not in _PROGRAM_CACHE:
        _PROGRAM_CACHE[key] = _build_program(W, NG, mm_dtype, epilogue)
    return _PROGRAM_CACHE[key]


def kernel(x, task_ids, l1_emb, l1_bias, l2_emb, l2_bias):
    global LAST_RESULTS
    x = np.ascontiguousarray(np.asarray(x, dtype=np.float32))
    tid = np.asarray(task_ids).astype(np.int64)
    l1_emb = np.ascontiguousarray(np.asarray(l1_emb, dtype=np.float32))
    l1_bias = np.ascontiguousarray(np.asarray(l1_bias, dtype=np.float32))
    l2_emb = np.ascontiguousarray(np.asarray(l2_emb, dtype=np.float32))
    l2_bias = np.ascontiguousarray(np.asarray(l2_bias, dtype=np.float32))

    B = x.shape[0]
    assert x.shape == (BATCH, N_FEATURES) and tid.shape == (BATCH,)

    if MM_DTYPE == "float8e3":
        import ml_dtypes

        wdt, mdt = ml_dtypes.float8_e3m4, np.float16
    elif MM_DTYPE == "float16":
        wdt = mdt = np.float16
    else:
        wdt = mdt = np.float32
    W = 8

    # A "group" is (task, slice of up to W of its samples).  Tasks with more
    # than W samples get several groups (their w1 row is duplicated in the
    # slab); tasks with no samples still get one group so that in the common
    # case the slab is exactly the core's contiguous l1_emb slice.
    counts = np.bincount(tid, minlength=NUM_TASKS)
    ngroups = np.maximum(1, -(-counts // W)).astype(np.int64)  # per task
    ng_core = ngroups.reshape(N_CORES, TPC).sum(axis=1)
    NG = int(ng_core.max())
    NSLOT = NG * W

    # within-core group base of each task
    gbase = np.empty(NUM_TASKS, dtype=np.int64)
    for c in range(N_CORES):
        sl = slice(c * TPC, (c + 1) * TPC)
        cs = np.cumsum(ngroups[sl])
        gbase[sl] = cs - ngroups[sl]

    # slot routing: sample j -> (core, slot)
    order = np.argsort(tid, kind="stable")
    sorted_tid = tid[order]
    starts = np.flatnonzero(np.r_[True, np.diff(sorted_tid) != 0])
    run_len = np.diff(np.r_[starts, B])
    run_pos = np.arange(B) - np.repeat(starts, run_len)
    occ = np.empty(B, dtype=np.int64)
    occ[order] = run_pos
    core = tid // TPC
    slot = (gbase[tid] + occ // W) * W + occ % W

    # scatter x into per-core transposed, padded slot grids, then repack as
    # [128, 2*NSLOT] (K-chunk 0 columns, then K-chunk 1 columns) so the
    # whole x rides one/two contiguous HWDGE transfers
    xT = np.zeros((N_CORES, N_FEATURES, NSLOT), dtype=mdt)
    xT[core, :, slot] = x.astype(mdt)

    inv = np.float32(INV_SQRT2)
    chunks = _dma_chunks(W, NG)
    in_maps = []
    for c in range(N_CORES):
        t0 = c * TPC
        sl = slice(t0, t0 + TPC)
        # task id of each group (padded to NG with the core's first task)
        gtask = np.repeat(np.arange(t0, t0 + TPC), ngroups[sl])
        if len(gtask) < NG:
            gtask = np.r_[gtask, np.full(NG - len(gtask), t0)]
        rows = l1_emb[gtask]  # [NG, 32768]
        # pack w1 per DMA chunk: [ct, 2, 128, 128] -> [128, ct, 2, 128] flat
        parts = []
        cum = 0
        for ct in chunks:
            blk = rows[cum:cum + ct]
            blk = blk.reshape(ct, 2, 128, 128).transpose(2, 0, 1, 3)
            parts.append(blk.astype(wdt).reshape(-1))
            cum += ct
        xc2 = xT[c].reshape(2, 128, NSLOT)
        in_maps.append({
            "xT": np.ascontiguousarray(
                np.concatenate([xc2[0], xc2[1]], axis=1)),
            "w1s": np.concatenate(parts),
            "b1Ts": np.ascontiguousarray(l1_bias[gtask].T)
            * (np.float32(1.0) if EPILOGUE == "gelu" else inv),
            "w2T": np.ascontiguousarray(l2_emb[gtask].T),
        })

    nc = _get_program(W, NG, MM_DTYPE, EPILOGUE)
    if SIM_CORES is not None:
        from concourse.bass_interp import CoreSim

        sim_results = []
        for c in range(N_CORES):
            if c in SIM_CORES:
                kw = {}
                if SIM_EXECUTOR_CLS is not None:
                    kw["executor_cls"] = SIM_EXECUTOR_CLS
                sim = CoreSim(nc, publish_trace=False, **kw)
                for k, v in in_maps[c].items():
                    sim.tensor(k)[:] = v
                sim.simulate()
                sim_results.append({"out": np.array(sim.tensor("out"))})
            else:
                sim_results.append({"out": np.zeros((1, NSLOT), np.float32)})
        outs = np.stack([r["out"].reshape(NSLOT) for r in sim_results])
        logits = outs[core, slot] + l2_bias[tid, 0]
        return logits[:, None].astype(np.float32)

    res = run_bass_kernel_spmd(
        nc, in_maps, core_ids=list(range(N_CORES)), trace=TRACE, tmpdir=TMPDIR,
    )
    LAST_RESULTS = res

    outs = np.stack([r["out"].reshape(NSLOT) for r in res.results])
    logits = outs[core, slot] + l2_bias[tid, 0]
    return logits[:, None].astype(np.float32)



# revision 23
# speedup vs baseline: 1.1289x; 1.0326x over previous
"""Trainium2 Bass kernel for the per-task embedding MLP (embedding_lookup).

Computation (per sample j with task t = task_ids[j]):
    h      = x[j] @ l1_emb[t].reshape(256, 128) + l1_bias[t]
    g      = gelu_exact(h)
    out[j] = sum(g * l2_emb[t]) + l2_bias[t, 0]

Strategy: shard the *task* axis across the 8 cores (125 contiguous tasks per
core), so each core streams a contiguous slab of l1_emb exactly once (the
memory roofline), instead of gathering a 128 KiB row per sample (4x more
traffic).  Samples are routed (host-side index math only) to the core owning
their task and packed into a fixed slot grid of W=8 columns per group (tasks
with more than W samples get extra groups with duplicated weight rows), so
all 8 cores run one identical SPMD program.  W=8 rather than 16: with ~4.1
samples/task the group count is nearly unchanged (so PE cost is the same)
but the x slot grid and the whole vector/scalar epilogue halve; only ~3
tasks/core overflow 8 samples and duplicate their 64 KiB w1 row.

Dataflow (v10):
  * The whole per-core w1 slab (NG tasks x 512 B/partition fp16, ~8 MiB)
    lives in ONE persistent SBUF tile.  All chunk DMAs are issued
    back-to-back on the sync HWDGE ring at kernel start (ramp [1, 8] so
    the first matmul starts ~1.5 us in, 32-task chunks in the middle —
    a finer ramp-in costs ~2 us of issue-rate idle — and a [16, 4, 2, 1]
    taper so the final matmuls are not gated on a megabyte receipt).  With no pool recycling the 16 DMA
    engines stream the slab at the ~370 GB/s per-core HBM ceiling and
    matmuls chase per-chunk completion semaphores (subtile deps).
  * x (fp16, [2,128,NSLOT]) and b1/w2 ride the gpsimd SWDGE queue so the
    sync ring carries nothing but the w1 stream.
  * Per PSUM block (<=16 groups): two K=128 fp16 matmuls per group
    accumulate hT[128, cols]; Vector does hs = ps + b1 (column-broadcast
    STT), Scalar does gelu (ACT table), Vector multiplies by w2 into a
    per-SEGMENT fp16 prodt tile.  The hidden-dim reduction (ones-vector
    matmul) runs once per <=512-col SEGMENT, not per block — a per-block
    reduce would make the in-order PE stream wait on the full
    STT->gelu->TT chain every block (~2.6 us/block convoy, the dominant
    cost of earlier revisions).  Segment results are copied PSUM->SBUF on
    the scalar engine and DMA'd out on the (by then drained) sync ring;
    the copy+DMA of segment s are emitted inside segment s+1 so no
    engine's in-order stream ever stalls on them.
  * l2_bias is added on the host (per-sample scalar add, free in numpy).

The stage-1 matmul operands (x, w1) are cast to fp16 on the host: fp32
matmuls on trn2 lower to LOW/HIGH double passes (~460 ns/task measured vs
~150 ns for fp16) and fp16 also halves the dominant l1_emb DMA traffic.
Accumulation (PSUM) and the whole epilogue stay fp32; measured end-to-end
L2 relative error ~3.6e-4.

Measured (core 0 NTFF, 8-core SPMD): ~43.0-45.3 us vs 46.3-48.3 us for the
previous pool-based kernel under identical conditions (run-to-run spread is
+-1.5 us from 8-core HBM contention).  Breakdown: ~2.5 us lead-in (preamble
barrier to first w1 byte), ~23 us w1 stream at the ~370 GB/s per-core HBM
ceiling (apparent "gaps" in the w1 packet timeline are x/const SWDGE
transfers sharing the same 16 DMA engines — only ~1.5 us is true idle),
~4.5 us tail drain (last chunk's matmuls + STT/gelu/TT chain + copy + out
DMA), ~9.3 us fixed runtime semaphore-file reset (present for any kernel,
outside the NEFF — each engine clears its 51-sem file one instruction at a
time).
"""

import numpy as np

import concourse.bacc as bacc
import concourse.mybir as mybir
import concourse.tile as tile
from concourse.bass_utils import run_bass_kernel_spmd

NUM_TASKS = 1000
N_FEATURES = 256
HIDDEN = 128
BATCH = 4096
N_CORES = 8
TPC = NUM_TASKS // N_CORES  # tasks per core = 125

INV_SQRT2 = float(1.0 / np.sqrt(2.0))

# Module-level knobs for the test harness (the grader just calls kernel()).
# MM_DTYPE: dtype of the stage-1 weights (l1_emb slab) fed to the PE.
#   "float8e3"  - w1 in e3m4 fp8 (4 mantissa bits), x stays fp16; halves the
#                 dominant w1 DMA stream.  Measured L2 rel err ~1.3e-2 (the
#                 harness gate is 2e-2); the PE upconverts both operands
#                 internally so mixed fp8 x fp16 matmuls are native.
#   "float16"   - w1 and x in fp16 (L2 rel err ~3.6e-4)
#   "float32"   - exact fallback
MM_DTYPE = "float8e3"
EPILOGUE = "gelu"  # "gelu" (ACT Gelu table) or "erf" (0.5x(1+erf(x/sqrt2)))
TRACE = False
TMPDIR = None  # optional fixed artifact dir for profiling runs
SIM_CORES = None  # e.g. [0]: run CoreSim for those cores instead of hardware
SIM_EXECUTOR_CLS = None  # optional InstructionExecutor subclass for CoreSim
LAST_RESULTS = None

_PROGRAM_CACHE = {}


def _ramp(total, head, tail, mid):
    """Sizes summing to `total`: `head` ramp-in, `tail` ramp-out, `mid`-sized
    chunks between.  Head/tail entries are dropped (largest first) if total
    is too small to fit them."""
    head = list(head)
    tail = list(tail)
    while head and sum(head) + sum(tail) > total:
        head.pop()
    while tail and sum(head) + sum(tail) > total:
        tail.pop()
    rem = total - sum(head) - sum(tail)
    assert rem >= 0
    mids = [mid] * (rem // mid) + ([rem % mid] if rem % mid else [])
    sizes = head + mids + tail
    assert sum(sizes) == total and all(s > 0 for s in sizes)
    return sizes


def _block_sizes(W, NG):
    """PSUM block sizes (in groups).  Uniform moderate blocks so the
    PE -> Vector -> Scalar pipeline advances in fine steps (big blocks
    turn the tail into a latency convoy); small head for an early epilogue
    start, small tail for a short post-last-matmul chain."""
    assert 512 % W == 0
    gmax = 512 // W
    sizes = _ramp(NG, [4, 8], [4, 2, 1], min(16, gmax))
    assert all(s <= gmax for s in sizes)
    return sizes


def _dma_chunks(W, NG):
    """w1 DMA chunk sizes (in tasks).  The [4, 8] head keeps the sync ring's
    first transfers SMALL: the 16 DMA engines service the sync ring (q1)
    preferentially, and an 8 KiB-line flood from t=8 us starves the scalar
    ring's x transfer until ~11-18 us (measured), gating every matmul.  With
    a small head, x streams out in the gap before the first 32-task chunk
    saturates the engines.  A short taper at the end keeps the last matmuls
    from waiting on a megabyte receipt."""
    return _ramp(NG, [4, 8], [8, 4, 2, 1], 32)


def _build_program(W, NG, mm_dtype, epilogue):
    """Emit the SPMD Tile program for slot width W and NG groups per core."""
    chunks = _dma_chunks(W, NG)
    sizes = _block_sizes(W, NG)
    use_gelu = epilogue == "gelu"
    NSLOT = NG * W
    f32 = mybir.dt.float32
    # wdt: stage-1 weight dtype (what the w1 slab is stored/streamed as).
    # mdt: dtype of x and of the fp16 epilogue tiles (prodt, cones) — stays
    # fp16 when the weights drop to fp8 (e3m4 prodt would overflow at ~15.5
    # and the PE takes mixed-dtype operands natively).
    wdt = getattr(mybir.dt, mm_dtype)
    mdt = mybir.dt.float16 if mm_dtype == "float8e3" else wdt

    nc = bacc.Bacc("TRN2", target_bir_lowering=False, debug=False)

    xT_d = nc.dram_tensor("xT", [128, 2 * NSLOT], mdt, kind="ExternalInput").ap()
    # w1 slab, host-packed per DMA chunk in partition-major [128, ct, 2, 128]
    # layout, one contiguous region per chunk
    w1_d = nc.dram_tensor(
        "w1s", [NG * N_FEATURES * HIDDEN], wdt, kind="ExternalInput"
    ).ap()
    # b1 and w2 host-packed into one fp16 [128, 2*NG] tile (b1 cols then w2
    # cols): a single HWDGE transfer on the scalar ring right behind x
    bw_d = nc.dram_tensor("bwT", [128, 2 * NG], mdt, kind="ExternalInput").ap()
    out_d = nc.dram_tensor("out", [1, NSLOT], f32, kind="ExternalOutput").ap()

    act_fn = (
        mybir.ActivationFunctionType.Gelu
        if use_gelu
        else mybir.ActivationFunctionType.Erf
    )
    add = mybir.AluOpType.add
    mult = mybir.AluOpType.mult

    with tile.TileContext(nc) as tc:
        with (
            tc.tile_pool(name="const", bufs=1) as constp,
            tc.tile_pool(name="work", bufs=5) as workp,
            tc.tile_pool(name="hpsum", bufs=5, space="PSUM") as hpsp,
            tc.tile_pool(name="opsum", bufs=3, space="PSUM") as opsp,
        ):
            # The whole w1 slab lives in SBUF (NG*512 B per partition).
            # All chunk DMAs are issued upfront on the sync HWDGE ring —
            # nothing else rides that ring, so the 16 DMA engines stream
            # the slab back-to-back; matmuls chase per-chunk completion
            # semaphores (subtile deps).
            w1sl = constp.tile([128, NG, 2, 128], wdt)
            w1off = 0
            q0 = 0
            for ct in chunks:
                ln = 128 * ct * 2 * 128
                blk = w1_d[w1off:w1off + ln].rearrange(
                    "(p g c h) -> p g c h", p=128, g=ct, c=2
                )
                nc.sync.dma_start(out=w1sl[:, q0:q0 + ct], in_=blk)
                w1off += ln
                q0 += ct

            # x columns, transposed, host-packed as one [128, 2*NSLOT] tile
            # (cols 0..NSLOT-1 = K-chunk 0, cols NSLOT.. = K-chunk 1).  It
            # rides the SCALAR engine's HWDGE ring: a second hardware ring in
            # parallel with the sync ring's w1 stream.  (The previous SWDGE
            # route ran at ~51 GB/s and gated the whole matmul stream until
            # t=21 us; HWDGE moves the 0.5 MiB in ~1.5 us.)  Two DMAs so the
            # K-chunk-0 columns (which un-gate the first matmuls) don't wait
            # on the full transfer.
            xc = constp.tile([128, 2 * NSLOT], mdt)
            nc.scalar.dma_start(out=xc[:, :NSLOT], in_=xT_d[:, :NSLOT])
            nc.scalar.dma_start(out=xc[:, NSLOT:], in_=xT_d[:, NSLOT:])

            # b1/w2 follow x on the scalar HWDGE ring (the gpsimd SWDGE
            # route ran at ~51 GB/s and delivered b1 only at ~13 us,
            # stalling the first STT and the whole epilogue pipeline).
            bwT = constp.tile([128, 2 * NG], mdt)
            nc.scalar.dma_start(out=bwT, in_=bw_d)

            # fp16 ones-vector + fp16 product: the hidden-dim reduce matmul
            # runs single-pass (fp32 would take the ~1 us LOW/HIGH path and
            # stall the PE queue between blocks); accumulation stays fp32
            cones = constp.tile([128, 1], mdt)
            nc.vector.memset(cones, 1.0 if use_gelu else INV_SQRT2)

            # Per-block hidden-dim reductions land in a small PSUM ring,
            # are copied to an SBUF staging row on the (mostly idle) scalar
            # engine, and leave via a few merged SWDGE DMAs.  The +b2
            # happens on the host (per-sample scalar add, free in numpy).
            out_sb = constp.tile([1, NSLOT], f32)
            segs = []  # [first_block, cols] per output-DMA segment
            for b, gbt in enumerate(sizes):
                if segs and segs[-1][1] + gbt * W <= 512:
                    segs[-1][1] += gbt * W
                else:
                    segs.append([b, gbt * W])
            # The epilogue's binding constraint is the PE's in-order stream:
            # a per-block reduce matmul would make MMs(b+1) wait on the full
            # STT -> gelu -> TT chain of block b (~2.6 us/block convoy).
            # Instead the hidden-dim reduce runs once per SEGMENT (<=512
            # cols = up to 4 blocks): TTs write into a shared per-segment
            # prodt tile, and the PE pays one chain round-trip per segment.
            # The PSUM->SBUF copy and output DMA of segment s are emitted
            # inside segment s+1 (software pipelining) so the scalar
            # engine's in-order stream never stalls on them.
            def stt_gelu(b):
                g0, cols, base, ps, hs, esb = state[b]
                # One full-block STT + one full-block gelu: Vector and
                # Scalar are throughput-bound across the kernel (ACTIVATE's
                # ~300 ns fixed cost dominates), so the former half-block
                # split just doubled the per-block overhead.
                b1v = (
                    bwT[:, g0:g0 + sizes[b]]
                    .unsqueeze(2).broadcast_to([128, sizes[b], W])
                )
                # hs = h*s + b1*s (s = 1/sqrt2 for erf path, 1 for gelu;
                # bwT's b1 half is host-scaled by s)
                nc.vector.scalar_tensor_tensor(
                    hs.rearrange("p (g w) -> p g w", w=W),
                    ps.rearrange("p (g w) -> p g w", w=W),
                    1.0 if use_gelu else INV_SQRT2, b1v, op0=mult, op1=add,
                )
                nc.scalar.activation(esb, hs, act_fn)
                if not use_gelu:
                    # tt = (e + 1) * hs = sqrt(2) * gelu(h)  (in-place)
                    nc.vector.scalar_tensor_tensor(
                        esb, esb, 1.0, hs, op0=add, op1=mult,
                    )

            def tt(b, prodt_seg, seg_base_col):
                g0, cols, base, ps, hs, esb = state[b]
                # prod = g * w2 (column-broadcast), cast to fp16, into this
                # block's slice of the segment prodt tile
                w2v = (
                    bwT[:, NG + g0:NG + g0 + sizes[b]]
                    .unsqueeze(2).broadcast_to([128, sizes[b], W])
                )
                off = base - seg_base_col
                nc.vector.tensor_mul(
                    prodt_seg[:, off:off + cols].rearrange(
                        "p (g w) -> p g w", w=W),
                    esb.rearrange("p (g w) -> p g w", w=W),
                    w2v,
                )

            def reduce_seg(si):
                sbase, scols, prodt_seg = seg_state[si]
                # reduce over hidden: [1, scols] = cones.T @ prodt_seg
                ops = opsp.tile([1, scols], mybir.dt.float32, tag="ops")
                nc.tensor.matmul(
                    ops, lhsT=cones, rhs=prodt_seg, start=True, stop=True)
                seg_ops[si] = ops
                if si > 0:
                    copy_out_seg(si - 1)

            def copy_out_seg(si, last=False):
                sbase, scols, _ = seg_state.pop(si)
                ops = seg_ops.pop(si)
                # PSUM -> SBUF staging on the scalar engine (DMA can't read
                # PSUM directly).  A single merged output DMA follows the
                # LAST copy on the scalar engine's own HWDGE ring: one
                # ~650 ns issue instead of one per segment, directly after
                # the copy in scalar's in-order stream (no cross-engine
                # semaphore hop on the critical tail).
                nc.scalar.copy(out_sb[:, sbase:sbase + scols], ops)
                if last:
                    nc.scalar.dma_start(out=out_d, in_=out_sb)

            state = {}
            seg_state = {}  # si -> (col_base, cols, prodt tile)
            seg_ops = {}
            block_seg = {}  # block index -> si
            for si, (b0, scols) in enumerate(segs):
                nxt = segs[si + 1][0] if si + 1 < len(segs) else len(sizes)
                for b in range(b0, nxt):
                    block_seg[b] = si

            # A segment's reduce matmul is NOT emitted at the segment
            # boundary: the PE's in-order stream would then stall for the
            # full STT -> gelu -> TT chain of the segment's last block
            # (~1-2 us convoy, measured) before continuing with stage-1
            # matmuls it already has data for.  Instead the reduce is
            # emitted 2 blocks into the NEXT segment, giving the Vector/
            # Scalar pipeline two blocks of slack to finish the TTs while
            # the PE keeps streaming.  The previous segment's PSUM->SBUF
            # copy rides along at the same point (reduce_seg emits it).
            pending = []  # (si, emit-after-block)
            for b, gbt in enumerate(sizes):
                g0 = sum(sizes[:b])
                cols = gbt * W
                base = g0 * W
                si = block_seg[b]
                if b == segs[si][0]:  # first block of its segment
                    sbase = sum(s[1] for s in segs[:si])
                    prodt_seg = workp.tile(
                        [128, segs[si][1]], mdt, tag="prodt", name="prodt_seg")
                    seg_state[si] = (sbase, segs[si][1], prodt_seg)

                ps = hpsp.tile([128, cols], mybir.dt.float32, tag="hps")
                for jj in range(gbt):
                    sl = slice(jj * W, (jj + 1) * W)
                    xlo = base + jj * W
                    nc.tensor.matmul(
                        ps[:, sl], lhsT=w1sl[:, g0 + jj, 0],
                        rhs=xc[:, xlo:xlo + W],
                        start=True, stop=False,
                    )
                    nc.tensor.matmul(
                        ps[:, sl], lhsT=w1sl[:, g0 + jj, 1],
                        rhs=xc[:, NSLOT + xlo:NSLOT + xlo + W],
                        start=False, stop=True,
                    )
                for si_, due in list(pending):
                    if b >= due:
                        reduce_seg(si_)
                        pending.remove((si_, due))
                hs = workp.tile([128, cols], f32, tag="hs")
                esb = workp.tile([128, cols], f32, tag="esb")
                state[b] = (g0, cols, base, ps, hs, esb)
                stt_gelu(b)
                tt(b, seg_state[si][2], seg_state[si][0])
                if b + 1 >= len(sizes) or block_seg[b + 1] != si:
                    pending.append((si, b + 2))
            for si_, due in pending:
                reduce_seg(si_)
            copy_out_seg(len(segs) - 1, last=True)


    nc.compile()
    return nc


def _get_program(W, NG, mm_dtype, epilogue):
    key = (W, NG, mm_dtype, epilogue)
    if key not in _PROGRAM_CACHE:
        _PROGRAM_CACHE[key] = _build_program(W, NG, mm_dtype, epilogue)
    return _PROGRAM_CACHE[key]


def kernel(x, task_ids, l1_emb, l1_bias, l2_emb, l2_bias):
    global LAST_RESULTS
    x = np.ascontiguousarray(np.asarray(x, dtype=np.float32))
    tid = np.asarray(task_ids).astype(np.int64)
    l1_emb = np.ascontiguousarray(np.asarray(l1_emb, dtype=np.float32))
    l1_bias = np.ascontiguousarray(np.asarray(l1_bias, dtype=np.float32))
    l2_emb = np.ascontiguousarray(np.asarray(l2_emb, dtype=np.float32))
    l2_bias = np.ascontiguousarray(np.asarray(l2_bias, dtype=np.float32))

    B = x.shape[0]
    assert x.shape == (BATCH, N_FEATURES) and tid.shape == (BATCH,)

    if MM_DTYPE == "float8e3":
        import ml_dtypes

        wdt, mdt = ml_dtypes.float8_e3m4, np.float16
    elif MM_DTYPE == "float16":
        wdt = mdt = np.float16
    else:
        wdt = mdt = np.float32
    W = 8

    # A "group" is (task, slice of up to W of its samples).  Tasks with more
    # than W samples get several groups (their w1 row is duplicated in the
    # slab); tasks with no samples still get one group so that in the common
    # case the slab is exactly the core's contiguous l1_emb slice.
    counts = np.bincount(tid, minlength=NUM_TASKS)
    ngroups = np.maximum(1, -(-counts // W)).astype(np.int64)  # per task
    ng_core = ngroups.reshape(N_CORES, TPC).sum(axis=1)
    NG = int(ng_core.max())
    NSLOT = NG * W

    # within-core group base of each task
    gbase = np.empty(NUM_TASKS, dtype=np.int64)
    for c in range(N_CORES):
        sl = slice(c * TPC, (c + 1) * TPC)
        cs = np.cumsum(ngroups[sl])
        gbase[sl] = cs - ngroups[sl]

    # slot routing: sample j -> (core, slot)
    order = np.argsort(tid, kind="stable")
    sorted_tid = tid[order]
    starts = np.flatnonzero(np.r_[True, np.diff(sorted_tid) != 0])
    run_len = np.diff(np.r_[starts, B])
    run_pos = np.arange(B) - np.repeat(starts, run_len)
    occ = np.empty(B, dtype=np.int64)
    occ[order] = run_pos
    core = tid // TPC
    slot = (gbase[tid] + occ // W) * W + occ % W

    # scatter x into per-core transposed, padded slot grids, then repack as
    # [128, 2*NSLOT] (K-chunk 0 columns, then K-chunk 1 columns) so the
    # whole x rides one/two contiguous HWDGE transfers
    xT = np.zeros((N_CORES, N_FEATURES, NSLOT), dtype=mdt)
    xT[core, :, slot] = x.astype(mdt)

    inv = np.float32(INV_SQRT2)
    chunks = _dma_chunks(W, NG)
    in_maps = []
    for c in range(N_CORES):
        t0 = c * TPC
        sl = slice(t0, t0 + TPC)
        # task id of each group (padded to NG with the core's first task)
        gtask = np.repeat(np.arange(t0, t0 + TPC), ngroups[sl])
        if len(gtask) < NG:
            gtask = np.r_[gtask, np.full(NG - len(gtask), t0)]
        rows = l1_emb[gtask]  # [NG, 32768]
        # pack w1 per DMA chunk: [ct, 2, 128, 128] -> [128, ct, 2, 128] flat
        parts = []
        cum = 0
        for ct in chunks:
            blk = rows[cum:cum + ct]
            blk = blk.reshape(ct, 2, 128, 128).transpose(2, 0, 1, 3)
            parts.append(blk.astype(wdt).reshape(-1))
            cum += ct
        xc2 = xT[c].reshape(2, 128, NSLOT)
        b1s = l1_bias[gtask].T * (
            np.float32(1.0) if EPILOGUE == "gelu" else inv)
        in_maps.append({
            "xT": np.ascontiguousarray(
                np.concatenate([xc2[0], xc2[1]], axis=1)),
            "w1s": np.concatenate(parts),
            "bwT": np.ascontiguousarray(np.concatenate(
                [b1s, l2_emb[gtask].T], axis=1).astype(mdt)),
        })

    nc = _get_program(W, NG, MM_DTYPE, EPILOGUE)
    if SIM_CORES is not None:
        from concourse.bass_interp import CoreSim

        sim_results = []
        for c in range(N_CORES):
            if c in SIM_CORES:
                kw = {}
                if SIM_EXECUTOR_CLS is not None:
                    kw["executor_cls"] = SIM_EXECUTOR_CLS
                sim = CoreSim(nc, publish_trace=False, **kw)
                for k, v in in_maps[c].items():
                    sim.tensor(k)[:] = v
                sim.simulate()
                sim_results.append({"out": np.array(sim.tensor("out"))})
            else:
                sim_results.append({"out": np.zeros((1, NSLOT), np.float32)})
        outs = np.stack([r["out"].reshape(NSLOT) for r in sim_results])
        logits = outs[core, slot] + l2_bias[tid, 0]
        return logits[:, None].astype(np.float32)

    res = run_bass_kernel_spmd(
        nc, in_maps, core_ids=list(range(N_CORES)), trace=TRACE, tmpdir=TMPDIR,
    )
    LAST_RESULTS = res

    outs = np.stack([r["out"].reshape(NSLOT) for r in res.results])
    logits = outs[core, slot] + l2_bias[tid, 0]
    return logits[:, None].astype(np.float32)

